# revision 30
# baseline (speedup 1.0000x reference)
"""GQA (grouped-query attention) Trainium2 kernel, tensor-parallel over 8 cores.

Problem: hidden [1,2048,4096] x (Wq[4096,4096], Wk/Wv[4096,1024], Wo[4096,4096])
H=32 query heads, G=8 KV groups, D=128, causal, RoPE (LLaMA rotate-half).

Sharding: core i owns query heads 4i..4i+3 and KV group i (Wq/Wk/Wv column
slices), plus the matching Wo row slice. Each core computes a full [2048,4096]
partial output; the host sums the 8 partials and adds bo.

On-device layout is fully transposed (partition = feature dim) so every matmul
runs with a 512-wide bf16 moving operand at full PE rate:
  phase A: single pass over X.T streaming 4-chunk tiles; each chunk feeds
           Q(x4)/K/V matmuls into 6 accumulating PSUM banks; RoPE fused.
           A(3)'s RoPE chains (needed only by B(3)) are deferred until after
           B(0) so the A->B transition isn't gated by the DVE queue.
  phase B: S.T = K.T x Q slabs -> exp -> P.T; ctx.T = V.T-chunks @ P.T;
           softmax denominator via fp32 running sums of P.T split across DVE
           and GpSimd plus a single ones-matmul per (head, seq-tile)
           (partition reduction + partition broadcast on PE). Diagonal
           key-blocks run first so the slow GpSimd chain drains early.
  phase C: out = ctx.T-chunks.T @ Wo rows, streamed back to DRAM; C(st) is
           emitted right after B(st+1) so its matmuls backfill PE gaps left
           by B's exp latency.

PSUM: one pool, 4 tags x 2 banks, shared between phases (A's k/v banks become
B's score banks the moment the bias-adds drain them, etc).
"""

import math

import numpy as np

import concourse.bacc as bacc
import concourse.tile as tile
from concourse import mybir
from concourse import bass_utils

# ---- problem constants (hardcoded per contest contract) ----
S = 2048          # sequence length
HID = 4096        # hidden size
H = 32            # query heads
G = 8             # KV groups
D = 128           # head dim
THETA = 10000.0
NCORES = 8
RH = H // NCORES      # query heads per core = 4
HD_LOC = RH * D       # local head width = 512

P = 128               # partitions
SQT = 512             # seq tile width (moving operand)
NSQT = S // SQT       # 4
NCH = HID // P        # 32 contraction chunks
NSKB = S // P         # 16 key blocks

f32 = mybir.dt.float32
f32r = mybir.dt.bfloat16
import ml_dtypes
bfnp = ml_dtypes.bfloat16

_SCALE = 1.0 / math.sqrt(D)
_NEG = -1.0e6


def _host_consts():
    """RoPE tables (transposed layout) + diag-mask + ones for partition sums."""
    half = D // 2
    inv_freq = 1.0 / (THETA ** (np.arange(half, dtype=np.float64) / half))
    ang = np.arange(S, dtype=np.float64)[None, :] * inv_freq[:, None]  # [64, S]
    cos = np.cos(ang)
    sin = np.sin(ang)
    cos_t = np.concatenate([cos, cos], axis=0).astype(np.float32).astype(bfnp)
    sin_t = np.concatenate([sin, -sin], axis=0).astype(np.float32).astype(bfnp)
    negut = np.triu(np.ones((P, P), np.float32), k=1) * _NEG
    ones_pp = np.ones((P, P), np.float32)
    return cos_t, sin_t, negut, ones_pp


def _round_f32r(x):
    return np.ascontiguousarray(np.asarray(x, np.float32)).astype(bfnp)


def build_kernel() -> bacc.Bacc:
    nc = bacc.Bacc("TRN2", target_bir_lowering=False, debug=False)

    xt = nc.dram_tensor("xt", [HID, S], f32r, kind="ExternalInput")
    wq = nc.dram_tensor("wq", [HID, HD_LOC], f32r, kind="ExternalInput")
    wk = nc.dram_tensor("wk", [HID, D], f32r, kind="ExternalInput")
    wv = nc.dram_tensor("wv", [HID, D], f32r, kind="ExternalInput")
    wo = nc.dram_tensor("wo", [HD_LOC, HID], f32r, kind="ExternalInput")
    bq = nc.dram_tensor("bq", [RH, D], f32, kind="ExternalInput")
    bk = nc.dram_tensor("bk", [1, D], f32, kind="ExternalInput")
    bv = nc.dram_tensor("bv", [1, D], f32, kind="ExternalInput")
    cos_d = nc.dram_tensor("cos_t", [P, S], f32r, kind="ExternalInput")
    sin_d = nc.dram_tensor("sin_t", [P, S], f32r, kind="ExternalInput")
    negut_d = nc.dram_tensor("negut", [P, P], f32r, kind="ExternalInput")
    onec_d = nc.dram_tensor("ones_pp", [P, P], f32r, kind="ExternalInput")
    out_d = nc.dram_tensor("out_partial", [S, HID], f32r, kind="ExternalOutput")

    Exp = mybir.ActivationFunctionType.Exp

    with tile.TileContext(nc) as tc:
        with tc.tile_pool(name="consts", bufs=1) as consts, \
             tc.tile_pool(name="qkv", bufs=1) as qkv, \
             tc.tile_pool(name="ps8", bufs=1, space="PSUM") as ps8, \
             tc.tile_pool(name="wa", bufs=1) as wa, \
             tc.tile_pool(name="xts", bufs=4) as xts, \
             tc.tile_pool(name="tmpa", bufs=2) as tmpa, \
             tc.tile_pool(name="ctxp", bufs=1) as ctxp, \
             tc.tile_pool(name="woc", bufs=1) as woc, \
             tc.tile_pool(name="outs", bufs=5) as outsp, \
             tc.tile_pool(name="ptp", bufs=8) as ptp, \
             tc.tile_pool(name="accp", bufs=2) as accp, \
             tc.tile_pool(name="tmpb", bufs=3) as tmpb:
            cos_sb = consts.tile([P, S], f32r)
            sin_sb = consts.tile([P, S], f32r)
            negut_sb = consts.tile([P, P], f32r)
            onec_sb = consts.tile([P, P], f32r)
            bq_sb = consts.tile([P, RH], f32)
            bk_sb = consts.tile([P, 1], f32)
            bv_sb = consts.tile([P, 1], f32)
            ident = consts.tile([P, P], f32r)

            # persistent transposed activations (bf16, matmul-ready)
            qt_sb = [qkv.tile([P, S], f32r, tag=f"qt{ob}", name=f"qt_sb{ob}") for ob in range(RH)]
            kt_sb = qkv.tile([P, S], f32r, tag="kt")
            v_sb = [qkv.tile([P, P], f32r, tag=f"v{skb}", name=f"v_sb{skb}")
                    for skb in range(NSKB)]
            vbt_sb = qkv.tile([P, S], f32r, tag="vbt")
            ctx_sb = [ctxp.tile([P, S], f32r, tag=f"ctx{hb}", name=f"ctx_sb{hb}") for hb in range(RH)]
            wo_sb = woc.tile([P, RH, HID], f32r)

            wa_q = wa.tile([P, NCH, HD_LOC], f32r)
            wa_k = wa.tile([P, NCH, D], f32r)
            wa_v = wa.tile([P, NCH, D], f32r)
            # per-chunk weight loads, interleaved so chunk c is ready when
            # the c-th matmul group wants it (first MM ~2us after preamble)
            wq_r = wq[:].rearrange("(c p) o -> p c o", p=P)
            wk_r = wk[:].rearrange("(c p) o -> p c o", p=P)
            wv_r = wv[:].rearrange("(c p) o -> p c o", p=P)
            for g in range(NCH // 4):
                cs = slice(4 * g, 4 * g + 4)
                nc.sync.dma_start(out=wa_k[:, cs], in_=wk_r[:, cs])
                nc.sync.dma_start(out=wa_v[:, cs], in_=wv_r[:, cs])
                for j in range(4):
                    c = 4 * g + j
                    nc.sync.dma_start(out=wa_q[:, c], in_=wq_r[:, c])
                if g == 2:
                    # RoPE tables + small consts: needed first at A(0)'s tail,
                    # so they queue behind the startup-critical weight chunks
                    nc.sync.dma_start(out=cos_sb, in_=cos_d[:])
                    nc.sync.dma_start(out=sin_sb, in_=sin_d[:])
                    nc.sync.dma_start(out=negut_sb, in_=negut_d[:])
                    nc.sync.dma_start(out=onec_sb, in_=onec_d[:])
                    nc.sync.dma_start(out=bq_sb, in_=bq[:].rearrange("o p -> p o"))
                    nc.sync.dma_start(out=bk_sb, in_=bk[:].rearrange("o p -> p o"))
                    nc.sync.dma_start(out=bv_sb, in_=bv[:].rearrange("o p -> p o"))
            from concourse.masks import make_identity
            make_identity(nc, ident)

            xt_g = xt[:].rearrange("(c p) s -> p c s", p=P)

            def rope_rot(dst, b, sq):
                t2 = tmpa.tile([P, SQT], f32r, tag="t2", name="rope_t2")
                nc.vector.tensor_mul(t2[0:64], b[64:128], sin_sb[64:128, sq])
                nc.vector.tensor_mul(t2[64:128], b[0:64], sin_sb[0:64, sq])
                t3 = tmpa.tile([P, SQT], f32r, tag="t3", name="rope_t3")
                nc.vector.tensor_mul(t3, b, cos_sb[:, sq])
                nc.vector.tensor_add(dst, t3, t2)

            def emit_A_mm(st, qsel=tuple(range(RH))):
                """Projection matmuls for seq-tile st + the PSUM-draining
                bias-adds (emitted first so the banks recycle promptly).
                Returns closures for the deferrable RoPE / V-transpose work.
                q heads not in qsel are skipped (deferred extra pass)."""
                sq = slice(st * SQT, (st + 1) * SQT)
                k_ps = ps8.tile([P, SQT], f32, tag="kv", bufs=2, name="k_ps")
                v_ps = ps8.tile([P, SQT], f32, tag="kv", bufs=2, name="v_ps")
                q_ps = {ob: ps8.tile([P, SQT], f32, tag=("qA" if ob < 2 else "qB"),
                                     bufs=(2 if ob < 2 else 3), name=f"q_ps{ob}")
                        for ob in qsel}
                for g in range(NCH // 4):
                    xt4 = xts.tile([P, 4, SQT], f32r, tag="xt4")
                    nc.scalar.dma_start(out=xt4, in_=xt_g[:, 4 * g:4 * g + 4, sq])
                    for j in range(4):
                        c = 4 * g + j
                        nc.tensor.matmul(k_ps, lhsT=wa_k[:, c, :], rhs=xt4[:, j],
                                         start=(c == 0), stop=(c == NCH - 1))
                        nc.tensor.matmul(v_ps, lhsT=wa_v[:, c, :], rhs=xt4[:, j],
                                         start=(c == 0), stop=(c == NCH - 1))
                        for ob in qsel:
                            nc.tensor.matmul(q_ps[ob],
                                             lhsT=wa_q[:, c, ob * P:(ob + 1) * P],
                                             rhs=xt4[:, j],
                                             start=(c == 0), stop=(c == NCH - 1))
                kb = tmpa.tile([P, SQT], f32r, tag="rk", name="rope_kb")
                nc.vector.tensor_scalar_add(kb, k_ps, bk_sb)
                nc.vector.tensor_scalar_add(vbt_sb[:, sq], v_ps, bv_sb)
                qb = {}
                for ob in qsel:
                    b = tmpa.tile([P, SQT], f32r, tag=f"rq{ob}", name=f"rope_qb{ob}")
                    nc.vector.tensor_scalar_add(b, q_ps[ob], bq_sb[:, ob:ob + 1])
                    qb[ob] = b

                def rope_k():
                    rope_rot(kt_sb[:, sq], kb, sq)

                def rope_q(ob):
                    rope_rot(qt_sb[ob][:, sq], qb[ob], sq)

                def q_pass(obs):
                    """Deferred-q extra pass: re-streams xt on the sync
                    queue; chunk-group emitter + bias/rope finisher."""
                    qd_ps = {ob: ps8.tile([P, SQT], f32, tag="qB", bufs=3,
                                          name=f"qd_ps{ob}") for ob in obs}

                    def chunks(gs):
                        for g in gs:
                            xt4 = xts.tile([P, 4, SQT], f32r, tag="xt4")
                            nc.sync.dma_start(out=xt4,
                                              in_=xt_g[:, 4 * g:4 * g + 4, sq])
                            for j in range(4):
                                c = 4 * g + j
                                for ob in obs:
                                    nc.tensor.matmul(
                                        qd_ps[ob],
                                        lhsT=wa_q[:, c, ob * P:(ob + 1) * P],
                                        rhs=xt4[:, j],
                                        start=(c == 0), stop=(c == NCH - 1))

                    def finish():
                        for ob in obs:
                            b = tmpa.tile([P, SQT], f32r, tag=f"rq{ob}",
                                          name=f"rope_qb{ob}")
                            nc.vector.tensor_scalar_add(b, qd_ps[ob],
                                                        bq_sb[:, ob:ob + 1])
                            qb[ob] = b
                        for ob in obs:
                            rope_rot(qt_sb[ob][:, sq], qb[ob], sq)

                    return chunks, finish

                def vtrans():
                    # V.T -> V for this tile's 4 key blocks
                    for skb in range(4 * st, 4 * st + 4):
                        ks = slice(skb * P, (skb + 1) * P)
                        vt_ps = ps8.tile([P, P], f32r, tag="vt", bufs=1, name="vt_ps")
                        nc.tensor.transpose(vt_ps, vbt_sb[:, ks], ident)
                        nc.scalar.copy(v_sb[skb][:], vt_ps)

                return rope_k, rope_q, vtrans, q_pass

            def emit_S(hb, skb, st):
                t = max(0, skb * P - st * SQT)
                diag = skb * P >= st * SQT
                sp = ps8.tile([P, SQT], f32, tag="kv", bufs=2, name="st_ps")
                nc.tensor.matmul(sp[:, t:SQT], lhsT=kt_sb[:, skb * P:(skb + 1) * P],
                                 rhs=qt_sb[hb][:, st * SQT + t:(st + 1) * SQT],
                                 start=True, stop=not diag)
                if diag:  # add -1e6 on the causal triangle, in PSUM
                    nc.tensor.matmul(sp[:, t:t + P], lhsT=negut_sb, rhs=ident,
                                     start=False, stop=True)
                return sp, t

            def emit_B_head(st, hb):
                sq = slice(st * SQT, (st + 1) * SQT)
                nblk = 4 * st + 4
                # diagonal (narrow) key-blocks first: the GpSimd side of the
                # denominator accumulation finishes early in the head instead
                # of sitting on the head-tail critical path
                order = list(range(4 * st, nblk)) + list(range(0, 4 * st))
                if True:
                    ctx_ps = ps8.tile([P, SQT], f32, tag="qA", bufs=2, name="ctx_ps")
                    # two parallel fp32 running sums of the probs replace the
                    # per-block ones-matmul partition reduction: GpSimd
                    # (otherwise idle) owns the early blocks, DVE the rest;
                    # the merge casts to bf16 for the ones-MM
                    acc_g = accp.tile([P, SQT], f32, tag="acc_g", name="acc_g")
                    acc_v = None
                    pend = emit_S(hb, order[0], st)
                    for pos, skb in enumerate(order):
                        sp, t = pend
                        if pos + 1 < nblk:  # S one block ahead of exp
                            pend = emit_S(hb, order[pos + 1], st)
                        pt = ptp.tile([P, SQT], f32r, tag="pt", name="pt")
                        nc.scalar.activation(out=pt[:, t:SQT], in_=sp[:, t:SQT],
                                             func=Exp, scale=_SCALE)
                        nc.tensor.matmul(ctx_ps[:, t:SQT], lhsT=v_sb[skb][:],
                                         rhs=pt[:, t:SQT], start=(pos == 0), stop=(pos == nblk - 1))
                        if pos == 0:
                            nc.vector.tensor_copy(acc_g, pt)
                        elif pos <= min(5, nblk - 1):
                            nc.gpsimd.tensor_add(acc_g[:, t:SQT], acc_g[:, t:SQT],
                                                 pt[:, t:SQT])
                        elif pos == 6:
                            acc_v = accp.tile([P, SQT], f32, tag="acc_v", name="acc_v")
                            nc.vector.tensor_copy(acc_v, pt)
                        else:
                            nc.vector.tensor_add(acc_v, acc_v, pt)
                    den_b = tmpb.tile([P, SQT], f32r, tag="den_b", name="den_b")
                    if acc_v is None:
                        nc.vector.tensor_copy(den_b, acc_g)
                    else:
                        nc.vector.tensor_add(den_b, acc_v, acc_g)
                    den_ps = ps8.tile([P, SQT], f32, tag="vt", bufs=1, name="den_ps")
                    nc.tensor.matmul(den_ps, lhsT=onec_sb, rhs=den_b,
                                     start=True, stop=True)
                    recip_sb = tmpb.tile([P, SQT], f32, tag="recip", name="recip_sb")
                    nc.vector.reciprocal(recip_sb, den_ps)
                    nc.vector.tensor_mul(ctx_sb[hb][:, sq], ctx_ps, recip_sb)

            def emit_B(st):
                for hb in range(RH):
                    emit_B_head(st, hb)

            def emit_C(st):
                """Generator: one output-column-pair unit per next()."""
                for sqb in range(st * (SQT // P), (st + 1) * (SQT // P)):
                    for g in range(4):  # pairs of output column tiles share lhsT loads
                        o_ps = [ps8.tile([P, SQT], f32, tag="qB", name=f"o_ps{e}", bufs=3)
                                for e in range(2)]
                        for hc in range(RH):
                            for e in range(2):
                                et = 2 * g + e
                                nc.tensor.matmul(
                                    o_ps[e],
                                    lhsT=ctx_sb[hc][:, sqb * P:(sqb + 1) * P],
                                    rhs=wo_sb[:, hc, et * SQT:(et + 1) * SQT],
                                    start=(hc == 0), stop=(hc == RH - 1))
                        for e in range(2):
                            et = 2 * g + e
                            o_sb = outsp.tile([P, SQT], f32r, tag="o_sb", name="o_sb")
                            if e == 0:
                                nc.vector.tensor_copy(o_sb, o_ps[e])
                            else:
                                nc.scalar.copy(o_sb, o_ps[e])
                            nc.sync.dma_start(
                                out=out_d[sqb * P:(sqb + 1) * P, et * SQT:(et + 1) * SQT],
                                in_=o_sb)
                        yield

            # ---------------- emission schedule ----------------
            for st in range(3):
                rope_k, rope_q, vtrans, _ = emit_A_mm(st)
                rope_k()
                for ob in range(RH):
                    rope_q(ob)
                vtrans()
            # wo loads (4MB) ride the quiet DMA window during A(3)'s matmuls;
            # 8 column chunks so early C units unblock as slices land
            wo_r = wo[:].rearrange("(h p) e -> p h e", p=P)
            for et in range(HID // SQT):
                es = slice(et * SQT, (et + 1) * SQT)
                nc.sync.dma_start(out=wo_sb[:, :, es], in_=wo_r[:, :, es])
            rope_k3, rope_q3, vtrans3, q_pass3 = emit_A_mm(3, qsel=(0, 1))
            chunks23, finish23 = q_pass3((2, 3))
            # A(3)'s RoPE chains are only needed by B(3), and A(3)'s q2/q3
            # projections are deferred entirely: both interleave between
            # B(0)'s heads as PE/DVE backfill for the exp-latency-bound start
            emit_B_head(0, 0); rope_k3(); chunks23((0, 1))
            emit_B_head(0, 1); rope_q3(0); chunks23((2, 3))
            emit_B_head(0, 2); rope_q3(1); chunks23((4, 5))
            emit_B_head(0, 3); chunks23((6, 7)); finish23(); vtrans3()
            emit_B(1)
            for _ in emit_C(0):
                pass
            emit_B(2)
            for _ in emit_C(1):
                pass
            emit_B(3)
            for _ in emit_C(2):
                pass
            for _ in emit_C(3):
                pass

    nc.compile()
    return nc


_CACHE = {}


def _get_kernel():
    if "nc" not in _CACHE:
        _CACHE["nc"] = build_kernel()
    return _CACHE["nc"]


def kernel(hidden_states, Wq, bq, Wk, bk, Wv, bv, Wo, bo, _trace=False, _trace_kwargs=None):
    hs = np.asarray(hidden_states, dtype=np.float32)
    B = hs.shape[0]
    assert hs.shape == (B, S, HID) and B == 1
    x = hs.reshape(S, HID)

    xt_r = _round_f32r(x.T)                           # [HID, S]
    cos_t, sin_t, negut, ones_pp = _host_consts()

    in_maps = []
    for i in range(NCORES):
        qs = slice(i * HD_LOC, (i + 1) * HD_LOC)
        ks = slice(i * D, (i + 1) * D)
        in_maps.append({
            "xt": xt_r,
            "wq": _round_f32r(np.asarray(Wq)[:, qs]),
            "wk": _round_f32r(np.asarray(Wk)[:, ks]),
            "wv": _round_f32r(np.asarray(Wv)[:, ks]),
            "wo": _round_f32r(np.asarray(Wo)[qs, :]),
            "bq": np.ascontiguousarray(np.asarray(bq, dtype=np.float32)[qs].reshape(RH, D)),
            "bk": np.ascontiguousarray(np.asarray(bk, dtype=np.float32)[ks].reshape(1, D)),
            "bv": np.ascontiguousarray(np.asarray(bv, dtype=np.float32)[ks].reshape(1, D)),
            "cos_t": cos_t,
            "sin_t": sin_t,
            "negut": _round_f32r(negut),
            "ones_pp": _round_f32r(ones_pp),
            })

    nc = _get_kernel()
    res = bass_utils.run_bass_kernel_spmd(
        nc, in_maps, core_ids=list(range(NCORES)),
        trace=_trace, **(_trace_kwargs or {}))

    acc = np.zeros((S, HID), dtype=np.float64)
    for i in range(NCORES):
        acc += res.results[i]["out_partial"].astype(np.float64)
    acc += np.asarray(bo, dtype=np.float64)[None, :]
    out = acc.astype(np.float32).reshape(1, S, HID)
    if _trace:
        return out, res
    return out
